# revision 47
# baseline (speedup 1.0000x reference)
"""PointTransformerV3 windowed sparse attention on 8 Trainium2 cores.

Strategy: shard the P=256 serialized windows across 8 cores (32 windows
each).  The host does the serialization gather (feat[order]) when
building each core's shard; everything else runs on-device:

  - qkv projection (bias folded via an appended ones-row of f^T)
  - RPE bias: for each axis a and head h the 64x64 table
    M_ah[u, v] = rpe[clip(u - v)] turns the lookup into two chained
    matmuls against one-hot coordinate encodings:
        S^T[j, i] += onehot(gc_j)^T @ (M_ah @ onehot(gc_i))
    which lands the bias directly in the attention-logit PSUM
    accumulation (no [K,K,H] gather is ever materialized in HBM).
  - batch mask: +32*onehot5(b_j) . 32*onehot5(b_i) adds 1024 to
    same-batch logits; exp is taken as exp(S - 1024) so cross-batch
    pairs underflow to exactly 0 like the reference's -1000 mask.
  - P @ V~ with proj folded into V~ and an appended ones column to get
    the softmax denominator for free; per-partition reciprocal scale.
"""

import numpy as np
import ml_dtypes

import concourse.bass as bass
import concourse.bacc as bacc
import concourse.mybir as mybir
from concourse.tile import TileContext
from concourse.bass_utils import run_bass_kernel_spmd

BF16 = mybir.dt.bfloat16
F32 = mybir.dt.float32

N, C, H, K, D = 65432, 64, 4, 256, 16
POS_BND, RPE_NUM = 20, 41
PAD = (K - N % K) % K          # 104
NPAD = N + PAD                 # 65536
P = NPAD // K                  # 256 windows
NCORES = 8
PC = P // NCORES               # 32 windows per core
NR = NPAD // NCORES            # 8192 rows per core
GMAX = 64                      # grid coord range [0, 64)
NB = 5                         # batch ids 0..3 plus pad id 4
MASK = 32.0                    # 32*32 = 1024 added to same-batch logits

bf = ml_dtypes.bfloat16


def _build_program():
    nc = bacc.Bacc()

    ft_d = nc.dram_tensor("ft", [C + 1, NR], BF16, kind="ExternalInput")
    u01_d = nc.dram_tensor("u01", [2 * GMAX, NR], BF16, kind="ExternalInput")
    u2b_d = nc.dram_tensor("u2b", [GMAX + NB, NR], BF16, kind="ExternalInput")
    # rows 0:64 axis0 (cols 0:256) + axis2 (cols 256:512); rows 64:128 axis1
    mtab_d = nc.dram_tensor("mtab", [128, 2 * H * GMAX], BF16, kind="ExternalInput")
    # head h occupies partitions 32h..32h+16 (32-aligned for AP rules)
    wq_d = nc.dram_tensor("wq", [C + 1, 128], BF16, kind="ExternalInput")
    wk_d = nc.dram_tensor("wk", [C + 1, 128], BF16, kind="ExternalInput")
    wv_d = nc.dram_tensor("wv", [C + 1, H * C], BF16, kind="ExternalInput")
    pb_d = nc.dram_tensor("pb", [128, C], BF16, kind="ExternalInput")
    z_d = nc.dram_tensor("z", [NR, C], F32, kind="ExternalOutput")

    with TileContext(nc) as tc:
        with (
            tc.tile_pool(name="const", bufs=1) as cpool,
            tc.tile_pool(name="io", bufs=4) as iopool,
            tc.tile_pool(name="work", bufs=3) as wpool,
            tc.tile_pool(name="heads", bufs=12) as hpool,
            tc.tile_pool(name="psA", bufs=1, space="PSUM") as psA,
            tc.tile_pool(name="psB", bufs=2, space="PSUM") as psB,
        ):
            mtab_s = cpool.tile_from(mtab_d[:, :])
            wq_s = cpool.tile_from(wq_d[:, :])
            wk_s = cpool.tile_from(wk_d[:, :])
            wv_s = cpool.tile_from(wv_d[:, :])
            pb_s = cpool.tile_from(pb_d[:, :])
            nbias = cpool.tile([128, 1], F32)
            nc.vector.memset(nbias[:, :], -1024.0)
            # double-buffered q-blockdiag tiles; zeros written once, only the
            # 16-row head blocks are rewritten each patch
            qzt = []
            for i in range(2):
                t = cpool.tile([128, H * K], BF16, name=f"qzc_{i}", tag=f"qzc{i}")
                nc.vector.memset(t[:, :], 0.0)
                qzt.append(t)

            def mt(a, h):
                if a == 0:
                    return mtab_s[0:64, h * GMAX:(h + 1) * GMAX]
                if a == 1:
                    return mtab_s[64:128, h * GMAX:(h + 1) * GMAX]
                return mtab_s[0:64, K + h * GMAX:K + (h + 1) * GMAX]

            for p in range(PC):
                sl = slice(p * K, (p + 1) * K)

                ft_s = iopool.tile([C + 1, K], BF16, name=f"ft_{p}", tag="ft")
                nc.sync.dma_start(out=ft_s[:, :], in_=ft_d[:, sl])
                u01_s = iopool.tile([128, K], BF16, name=f"u01_{p}", tag="u01")
                nc.sync.dma_start(out=u01_s[:, :], in_=u01_d[:, sl])

                # shared lhsT for the second S contraction tile:
                # rows 0:64 U2, 64:69 mask onehot (same for all heads)
                l2_s = iopool.tile([69, K], BF16, name=f"l2_{p}", tag="l2")
                nc.sync.dma_start(out=l2_s[:, :], in_=u2b_d[:, sl])
                # rhs for the second S tile, all heads side by side; one
                # broadcast DMA fills the mask rows for all 4 heads
                rhs2b = wpool.tile([69, H * K], BF16, name=f"rhs2b_{p}", tag="rhs2b")
                nc.sync.dma_start(
                    out=rhs2b[64:69, :].rearrange("p (h x) -> p h x", h=H),
                    in_=u2b_d[GMAX:GMAX + NB, sl].rearrange(
                        "p (o x) -> p o x", o=1).to_broadcast((NB, H, K)))
                rhs2 = [rhs2b[:, h * K:(h + 1) * K] for h in range(H)]

                # q/k projections: kq[:, 0:256] = k^T, [:, 256:512] = q^T*scale
                kq_ps = psA.tile([128, 2 * K], F32, name=f"kq_{p}", tag="kq")
                nc.tensor.matmul(kq_ps[:, 0:K], wk_s[:, :], ft_s[:, :])
                nc.tensor.matmul(kq_ps[:, K:2 * K], wq_s[:, :], ft_s[:, :])
                kq_s = wpool.tile([128, 2 * K], BF16, name=f"kqs_{p}", tag="kqs")
                nc.scalar.copy(kq_s[:, :], kq_ps[:, :])
                # per-head q with other heads' rows zeroed: contract-128 qk
                # matmul against all-head k picks out the h block only
                qz_s = qzt[p % 2]
                for h in range(H):
                    hs = slice(h * 32, h * 32 + D)
                    nc.vector.tensor_copy(
                        qz_s[hs, h * K:(h + 1) * K], kq_s[hs, K:2 * K])

                # V~ = f @ Wv~^T laid out [j, (h, o)]
                vb_ps = psA.tile([128, 2 * K], F32, name=f"vb_{p}", tag="vb")
                for jb in range(2):
                    nc.tensor.matmul(
                        vb_ps[:, jb * K:(jb + 1) * K],
                        ft_s[:, jb * 128:(jb + 1) * 128],
                        wv_s[:, :],
                    )
                # vs[j, jb, h*65:h*65+64] = V~_h ; col h*65+64 = ones
                vs_s = wpool.tile([128, 2, H * 65], BF16, name=f"vs_{p}", tag="vs")
                for jb in range(2):
                    nc.vector.tensor_copy(
                        vs_s[:, jb, :].rearrange("p (h x) -> p h x", h=H)[:, :, 0:C],
                        vb_ps[:, jb * K:(jb + 1) * K].rearrange(
                            "p (h x) -> p h x", h=H),
                    )
                nc.vector.memset(
                    vs_s.rearrange("p b (h x) -> p b h x", h=H)[:, :, :, C], 1.0)

                # RPE gather-equivalent: g_h[v, i-block] via M tables
                rhs1 = []
                for h in range(H):
                    g_ps = psB.tile([128, 2 * K], F32, name=f"g_{p}_{h}", tag="g")
                    nc.tensor.matmul(g_ps[0:64, 0:K], mt(0, h), u01_s[0:64, :])
                    nc.tensor.matmul(g_ps[64:128, 0:K], mt(1, h), u01_s[64:128, :])
                    nc.tensor.matmul(g_ps[0:64, K:2 * K], mt(2, h), l2_s[0:64, :])
                    r1 = hpool.tile([128, K], BF16, name=f"rhs1_{p}_{h}", tag="rhs1")
                    nc.scalar.copy(r1[:, :], g_ps[:, 0:K])
                    nc.vector.tensor_copy(rhs2[h][0:64, :], g_ps[0:64, K:2 * K])
                    rhs1.append(r1)

                # attention logits S^T[j, i] per head, then P = exp(S - 1024)
                pmat = []
                for h in range(H):
                    s_ps = psB.tile([128, 2 * K], F32, name=f"s_{p}_{h}", tag="s")
                    for jb in range(2):
                        jsl = slice(jb * 128, (jb + 1) * 128)
                        osl = slice(jb * K, (jb + 1) * K)
                        nc.tensor.matmul(
                            s_ps[:, osl], u01_s[:, jsl], rhs1[h][:, :],
                            start=True, stop=False)
                        nc.tensor.matmul(
                            s_ps[:, osl], l2_s[:, jsl], rhs2[h][:, :],
                            start=False, stop=False)
                        nc.tensor.matmul(
                            s_ps[:, osl], kq_s[:, jsl],
                            qz_s[:, h * K:(h + 1) * K],
                            start=False, stop=True)
                    pm = hpool.tile([128, 2 * K], BF16, name=f"p_{p}_{h}", tag="pm")
                    nc.scalar.activation(
                        pm[:, :], s_ps[:, :], mybir.ActivationFunctionType.Exp,
                        bias=nbias[:, :], scale=1.0)
                    pmat.append(pm)

                # u[i, h*65 : h*65+65] = [P @ V~_h | rowsum]
                ub = []
                for ib in range(2):
                    u_ps = psB.tile([128, H * 65], F32, name=f"u_{p}_{ib}", tag="u")
                    for h in range(H):
                        for jb in range(2):
                            nc.tensor.matmul(
                                u_ps[:, h * 65:(h + 1) * 65],
                                pmat[h][:, jb * K + ib * 128: jb * K + ib * 128 + 128],
                                vs_s[:, jb, h * 65:(h + 1) * 65],
                                start=(jb == 0), stop=(jb == 1))
                    ub.append(u_ps)

                # z = sum_h u_h / s_h + pb (adds on the idle gpsimd engine)
                zz = wpool.tile([128, 2, C], F32, name=f"zz_{p}", tag="zz")
                for ib in range(2):
                    r_s = wpool.tile([128, H], F32, name=f"r_{p}_{ib}", tag="r")
                    nc.vector.reciprocal(
                        r_s[:, :],
                        ub[ib].rearrange("p (h x) -> p h x", h=H)[:, :, C])
                    ys = []
                    for h in range(H):
                        y = hpool.tile([128, C], BF16, name=f"y_{p}_{ib}_{h}",
                                       tag="y", bufs=8)
                        nc.vector.tensor_scalar_mul(
                            y[:, :], ub[ib][:, h * 65:h * 65 + C], r_s[:, h:h + 1])
                        ys.append(y)
                    t01 = wpool.tile([128, C], BF16, name=f"t01_{p}_{ib}", tag="t01")
                    t23 = wpool.tile([128, C], BF16, name=f"t23_{p}_{ib}", tag="t23")
                    nc.gpsimd.tensor_add(t01[:, :], ys[0][:, :], ys[1][:, :])
                    nc.gpsimd.tensor_add(t23[:, :], ys[2][:, :], ys[3][:, :])
                    t03 = wpool.tile([128, C], BF16, name=f"t03_{p}_{ib}", tag="t03")
                    nc.gpsimd.tensor_add(t03[:, :], t01[:, :], t23[:, :])
                    nc.gpsimd.tensor_add(zz[:, ib, :], t03[:, :], pb_s[:, :])
                nc.sync.dma_start(
                    out=z_d[p * K:(p + 1) * K, :].rearrange(
                        "(b i) c -> i b c", b=2),
                    in_=zz[:, :, :])
    nc.compile()
    return nc


def _host_prep(feat, qkv_w, qkv_b, proj_w, proj_b, rpe_table,
               order, grid_coord, batch, num_batches):
    scale = D ** -0.5
    order = np.asarray(order)

    f_ser = np.zeros((NPAD, C), np.float32)
    f_ser[:N] = np.asarray(feat, np.float32)[order]
    gc_ser = np.zeros((NPAD, 3), np.int64)
    gc_ser[:N] = np.asarray(grid_coord)[order]
    b_ser = np.full((NPAD,), int(num_batches), np.int64)
    b_ser[:N] = np.asarray(batch)

    # shared parameter-derived tensors
    qw = np.asarray(qkv_w, np.float32)
    qb = np.asarray(qkv_b, np.float32)
    pw = np.asarray(proj_w, np.float32)
    rpe = np.asarray(rpe_table, np.float32)

    wq64 = np.concatenate([qw[0:C].T, qb[None, 0:C]], 0) * scale     # [65, 64]
    wk64 = np.concatenate([qw[C:2 * C].T, qb[None, C:2 * C]], 0)     # [65, 64]
    wq = np.zeros((C + 1, 128), np.float32)
    wk = np.zeros((C + 1, 128), np.float32)
    for h in range(H):
        wq[:, 32 * h:32 * h + D] = wq64[:, D * h:D * (h + 1)]
        wk[:, 32 * h:32 * h + D] = wk64[:, D * h:D * (h + 1)]
    wv = np.zeros((C + 1, H * C), np.float32)
    for h in range(H):
        vh = qw[2 * C + h * D: 2 * C + (h + 1) * D]                  # [16, 64]
        bh = qb[2 * C + h * D: 2 * C + (h + 1) * D]
        ph = pw[:, h * D:(h + 1) * D]                                # [64, 16]
        wv[0:C, h * C:(h + 1) * C] = vh.T @ ph.T
        wv[C, h * C:(h + 1) * C] = bh @ ph.T

    u, v = np.arange(GMAX)[:, None], np.arange(GMAX)[None, :]
    duv = np.clip(u - v, -POS_BND, POS_BND) + POS_BND
    mtab = np.zeros((128, 2 * H * GMAX), np.float32)
    for h in range(H):
        mtab[0:64, h * GMAX:(h + 1) * GMAX] = rpe[duv, h]
        mtab[64:128, h * GMAX:(h + 1) * GMAX] = rpe[duv + RPE_NUM, h]
        mtab[0:64, K + h * GMAX:K + (h + 1) * GMAX] = rpe[duv + 2 * RPE_NUM, h]

    pb = np.broadcast_to(np.asarray(proj_b, np.float32), (128, C)).copy()

    iota64 = np.arange(GMAX)
    in_maps = []
    for c in range(NCORES):
        rs = slice(c * NR, (c + 1) * NR)
        ft = np.ones((C + 1, NR), np.float32)
        ft[0:C] = f_ser[rs].T
        gc_c = gc_ser[rs]                                            # [NR, 3]
        u01 = np.zeros((2 * GMAX, NR), np.float32)
        u01[0:GMAX] = gc_c[:, 0][None, :] == iota64[:, None]
        u01[GMAX:] = gc_c[:, 1][None, :] == iota64[:, None]
        u2b = np.zeros((GMAX + NB, NR), np.float32)
        u2b[0:GMAX] = gc_c[:, 2][None, :] == iota64[:, None]
        u2b[GMAX:] = MASK * (b_ser[rs][None, :] == np.arange(NB)[:, None])
        in_maps.append({
            "ft": ft.astype(bf), "u01": u01.astype(bf), "u2b": u2b.astype(bf),
            "mtab": mtab.astype(bf), "wq": wq.astype(bf), "wk": wk.astype(bf),
            "wv": wv.astype(bf), "pb": pb.astype(bf),
        })
    return in_maps


def kernel(feat, qkv_w, qkv_b, proj_w, proj_b, rpe_table,
           order, inverse, grid_coord, batch, num_batches, _state={}):
    in_maps = _host_prep(feat, qkv_w, qkv_b, proj_w, proj_b, rpe_table,
                         order, grid_coord, batch, num_batches)
    if "nc" not in _state:
        _state["nc"] = _build_program()
    res = run_bass_kernel_spmd(_state["nc"], in_maps, list(range(NCORES)))
    y_ser = np.concatenate([np.asarray(r["z"]) for r in res.results], 0)
    out = np.empty((N, C), np.float32)
    out[np.asarray(order)] = y_ser[:N]
    return out


# revision 57
# speedup vs baseline: 1.0599x; 1.0599x over previous
"""PointTransformerV3 windowed sparse attention on 8 Trainium2 cores.

Strategy: shard the P=256 serialized windows across 8 cores (32 windows
each).  The host does the serialization gather (feat[order]) when
building each core's shard; everything else runs on-device:

  - qkv projection (bias folded via an appended ones-row of f^T)
  - RPE bias: for each axis a and head h the 64x64 table
    M_ah[u, v] = rpe[clip(u - v)] turns the lookup into two chained
    matmuls against one-hot coordinate encodings:
        S^T[j, i] += onehot(gc_j)^T @ (M_ah @ onehot(gc_i))
    which lands the bias directly in the attention-logit PSUM
    accumulation (no [K,K,H] gather is ever materialized in HBM).
  - batch mask: +32*onehot5(b_j) . 32*onehot5(b_i) adds 1024 to
    same-batch logits; exp is taken as exp(S - 1024) so cross-batch
    pairs underflow to exactly 0 like the reference's -1000 mask.
  - P @ V~ with proj folded into V~ and an appended ones column to get
    the softmax denominator for free; per-partition reciprocal scale.
"""

import numpy as np
import ml_dtypes

import concourse.bass as bass
import concourse.bacc as bacc
import concourse.mybir as mybir
from concourse.tile import TileContext
from concourse.bass_utils import run_bass_kernel_spmd

BF16 = mybir.dt.bfloat16
FP8 = mybir.dt.float8e4
F32 = mybir.dt.float32

N, C, H, K, D = 65432, 64, 4, 256, 16
POS_BND, RPE_NUM = 20, 41
PAD = (K - N % K) % K          # 104
NPAD = N + PAD                 # 65536
P = NPAD // K                  # 256 windows
NCORES = 8
PC = P // NCORES               # 32 windows per core
NR = NPAD // NCORES            # 8192 rows per core
GMAX = 64                      # grid coord range [0, 64)
NB = 5                         # batch ids 0..3 plus pad id 4
MASK = 32.0                    # 32*32 = 1024 added to same-batch logits

bf = ml_dtypes.bfloat16


def _build_program():
    nc = bacc.Bacc()

    ft_d = nc.dram_tensor("ft", [C + 1, NR], BF16, kind="ExternalInput")
    u01_d = nc.dram_tensor("u01", [2 * GMAX, NR], BF16, kind="ExternalInput")
    u2b_d = nc.dram_tensor("u2b", [GMAX + NB, NR], BF16, kind="ExternalInput")
    udr_d = nc.dram_tensor("udr", [128, 2 * NR], FP8, kind="ExternalInput")
    sel_d = nc.dram_tensor("sel", [NB, GMAX], BF16, kind="ExternalInput")
    # rows 0:64 axis0 (cols 0:256) + axis2 (cols 256:512); rows 64:128 axis1
    mtab_d = nc.dram_tensor("mtab", [128, 2 * H * GMAX], BF16, kind="ExternalInput")
    # head h occupies partitions 32h..32h+16 (32-aligned for AP rules)
    wq_d = nc.dram_tensor("wq", [C + 1, 128], BF16, kind="ExternalInput")
    wk_d = nc.dram_tensor("wk", [C + 1, 128], BF16, kind="ExternalInput")
    wv_d = nc.dram_tensor("wv", [C + 1, H * C], BF16, kind="ExternalInput")
    pb_d = nc.dram_tensor("pb", [128, C], BF16, kind="ExternalInput")
    z_d = nc.dram_tensor("z", [NR, C], F32, kind="ExternalOutput")

    with TileContext(nc) as tc:
        with (
            tc.tile_pool(name="const", bufs=1) as cpool,
            tc.tile_pool(name="io", bufs=4) as iopool,
            tc.tile_pool(name="work", bufs=3) as wpool,
            tc.tile_pool(name="heads", bufs=12) as hpool,
            tc.tile_pool(name="psA", bufs=1, space="PSUM") as psA,
            tc.tile_pool(name="psB", bufs=2, space="PSUM") as psB,
        ):
            mtab_s = cpool.tile_from(mtab_d[:, :])
            wq_s = cpool.tile_from(wq_d[:, :])
            wk_s = cpool.tile_from(wk_d[:, :])
            wv_s = cpool.tile_from(wv_d[:, :])
            pb_s = cpool.tile_from(pb_d[:, :])
            sel_s = cpool.tile_from(sel_d[:, :])
            nbias = cpool.tile([128, 1], F32)
            nc.vector.memset(nbias[:, :], -1024.0)
            # double-buffered q-blockdiag tiles; zeros written once, only the
            # 16-row head blocks are rewritten each patch
            qzt = []
            for i in range(2):
                t = cpool.tile([128, H * K], BF16, name=f"qzc_{i}", tag=f"qzc{i}")
                nc.vector.memset(t[:, :], 0.0)
                qzt.append(t)

            def mt(a, h):
                if a == 0:
                    return mtab_s[0:64, h * GMAX:(h + 1) * GMAX]
                if a == 1:
                    return mtab_s[64:128, h * GMAX:(h + 1) * GMAX]
                return mtab_s[0:64, K + h * GMAX:K + (h + 1) * GMAX]

            for p in range(PC):
                sl = slice(p * K, (p + 1) * K)

                ft_s = iopool.tile([C + 1, K], BF16, name=f"ft_{p}", tag="ft")
                nc.sync.dma_start(out=ft_s[:, :], in_=ft_d[:, sl])
                u01_s = iopool.tile([128, K], BF16, name=f"u01_{p}", tag="u01")
                nc.sync.dma_start(out=u01_s[:, :], in_=u01_d[:, sl])

                # shared lhsT for the second S contraction tile:
                # rows 0:64 U2, 64:69 mask onehot (same for all heads)
                l2_s = iopool.tile([69, K], BF16, name=f"l2_{p}", tag="l2")
                nc.sync.dma_start(out=l2_s[:, :], in_=u2b_d[:, sl])
                # DR-interleaved one-hot weights for the merged S matmul
                udr_s = iopool.tile([128, 2, K], FP8, name=f"udr_{p}", tag="udr")
                nc.sync.dma_start(
                    out=udr_s[:, :, :],
                    in_=udr_d.rearrange("p (s n) -> p s n", s=2)[:, :, sl])
                ohb_s = iopool.tile([NB, K], BF16, name=f"ohb_{p}", tag="ohb")
                nc.sync.dma_start(out=ohb_s[:, :], in_=u2b_d[GMAX:GMAX + NB, sl])

                # q/k projections: kq[:, 0:256] = k^T, [:, 256:512] = q^T*scale
                kq_ps = psA.tile([128, 2 * K], F32, name=f"kq_{p}", tag="kq")
                nc.tensor.matmul(kq_ps[:, 0:K], wk_s[:, :], ft_s[:, :])
                nc.tensor.matmul(kq_ps[:, K:2 * K], wq_s[:, :], ft_s[:, :])
                kq_s = wpool.tile([128, 2 * K], BF16, name=f"kqs_{p}", tag="kqs")
                nc.scalar.copy(kq_s[:, :], kq_ps[:, :])
                # per-head q with other heads' rows zeroed: contract-128 qk
                # matmul against all-head k picks out the h block only
                qz_s = qzt[p % 2]
                for h in range(H):
                    hs = slice(h * 32, h * 32 + D)
                    nc.vector.tensor_copy(
                        qz_s[hs, h * K:(h + 1) * K], kq_s[hs, K:2 * K])

                # V~ = f @ Wv~^T laid out [j, (h, o)]
                vb_ps = psA.tile([128, 2 * K], F32, name=f"vb_{p}", tag="vb")
                for jb in range(2):
                    nc.tensor.matmul(
                        vb_ps[:, jb * K:(jb + 1) * K],
                        ft_s[:, jb * 128:(jb + 1) * 128],
                        wv_s[:, :],
                    )
                # vs[j, jb, h*65:h*65+64] = V~_h ; col h*65+64 = ones
                vs_s = wpool.tile([128, 2, H * 65], BF16, name=f"vs_{p}", tag="vs")
                for jb in range(2):
                    nc.vector.tensor_copy(
                        vs_s[:, jb, :].rearrange("p (h x) -> p h x", h=H)[:, :, 0:C],
                        vb_ps[:, jb * K:(jb + 1) * K].rearrange(
                            "p (h x) -> p h x", h=H),
                    )
                nc.vector.memset(
                    vs_s.rearrange("p b (h x) -> p b h x", h=H)[:, :, :, C], 1.0)

                # RPE gather-equivalent, laid out for DoubleRow pairing:
                # partition v: (G0[v], G2[v]); partition 64+v: (G1[v], mask[v])
                rdr = []
                for h in range(H):
                    g_ps = psB.tile([128, 2 * K], F32, name=f"g_{p}_{h}", tag="g")
                    nc.tensor.matmul(g_ps[0:64, 0:K], mt(0, h), u01_s[0:64, :])
                    nc.tensor.matmul(g_ps[64:128, 0:K], mt(1, h), u01_s[64:128, :])
                    nc.tensor.matmul(g_ps[0:64, K:2 * K], mt(2, h), l2_s[0:64, :])
                    nc.tensor.matmul(g_ps[64:128, K:2 * K], sel_s[:, :], ohb_s[:, :])
                    rd = hpool.tile([128, 2 * K], FP8, name=f"rdr_{p}_{h}", tag="rdr")
                    if h < 2:
                        nc.scalar.copy(rd[:, :], g_ps[:, :])
                    else:
                        nc.vector.tensor_copy(rd[:, :], g_ps[:, :])
                    rdr.append(rd)

                # attention logits S^T[j, i] per head, then P = exp(S - 1024)
                pmat = []
                for h in range(H):
                    s_ps = psB.tile([128, 2 * K], F32, name=f"s_{p}_{h}", tag="s")
                    for jb in range(2):
                        jsl = slice(jb * 128, (jb + 1) * 128)
                        osl = slice(jb * K, (jb + 1) * K)
                        nc.tensor.matmul(
                            s_ps[:, osl], udr_s[:, :, jsl],
                            rdr[h].rearrange("p (s x) -> p s x", s=2),
                            start=True, stop=False,
                            perf_mode=mybir.MatmulPerfMode.DoubleRow)
                        nc.tensor.matmul(
                            s_ps[:, osl], kq_s[:, jsl],
                            qz_s[:, h * K:(h + 1) * K],
                            start=False, stop=True)
                    pm = hpool.tile([128, 2 * K], BF16, name=f"p_{p}_{h}", tag="pm")
                    nc.scalar.activation(
                        pm[:, :], s_ps[:, :], mybir.ActivationFunctionType.Exp,
                        bias=nbias[:, :], scale=1.0)
                    pmat.append(pm)

                # u[i, h*65 : h*65+65] = [P @ V~_h | rowsum]
                ub = []
                for ib in range(2):
                    u_ps = psB.tile([128, H * 65], F32, name=f"u_{p}_{ib}", tag="u")
                    for h in range(H):
                        for jb in range(2):
                            nc.tensor.matmul(
                                u_ps[:, h * 65:(h + 1) * 65],
                                pmat[h][:, jb * K + ib * 128: jb * K + ib * 128 + 128],
                                vs_s[:, jb, h * 65:(h + 1) * 65],
                                start=(jb == 0), stop=(jb == 1))
                    ub.append(u_ps)

                # z = sum_h u_h / s_h + pb (adds on the idle gpsimd engine)
                zz = wpool.tile([128, 2, C], F32, name=f"zz_{p}", tag="zz")
                for ib in range(2):
                    r_s = wpool.tile([128, H], F32, name=f"r_{p}_{ib}", tag="r")
                    nc.vector.reciprocal(
                        r_s[:, :],
                        ub[ib].rearrange("p (h x) -> p h x", h=H)[:, :, C])
                    ys = []
                    for h in range(H):
                        y = hpool.tile([128, C], BF16, name=f"y_{p}_{ib}_{h}",
                                       tag="y", bufs=8)
                        nc.vector.tensor_scalar_mul(
                            y[:, :], ub[ib][:, h * 65:h * 65 + C], r_s[:, h:h + 1])
                        ys.append(y)
                    t01 = wpool.tile([128, C], BF16, name=f"t01_{p}_{ib}", tag="t01")
                    t23 = wpool.tile([128, C], BF16, name=f"t23_{p}_{ib}", tag="t23")
                    nc.gpsimd.tensor_add(t01[:, :], ys[0][:, :], ys[1][:, :])
                    nc.gpsimd.tensor_add(t23[:, :], ys[2][:, :], ys[3][:, :])
                    t03 = wpool.tile([128, C], BF16, name=f"t03_{p}_{ib}", tag="t03")
                    nc.gpsimd.tensor_add(t03[:, :], t01[:, :], t23[:, :])
                    nc.gpsimd.tensor_add(zz[:, ib, :], t03[:, :], pb_s[:, :])
                nc.sync.dma_start(
                    out=z_d[p * K:(p + 1) * K, :].rearrange(
                        "(b i) c -> i b c", b=2),
                    in_=zz[:, :, :])
    nc.compile()
    return nc


def _host_prep(feat, qkv_w, qkv_b, proj_w, proj_b, rpe_table,
               order, grid_coord, batch, num_batches):
    scale = D ** -0.5
    order = np.asarray(order)

    f_ser = np.zeros((NPAD, C), np.float32)
    f_ser[:N] = np.asarray(feat, np.float32)[order]
    gc_ser = np.zeros((NPAD, 3), np.int64)
    gc_ser[:N] = np.asarray(grid_coord)[order]
    b_ser = np.full((NPAD,), int(num_batches), np.int64)
    b_ser[:N] = np.asarray(batch)

    # shared parameter-derived tensors
    qw = np.asarray(qkv_w, np.float32)
    qb = np.asarray(qkv_b, np.float32)
    pw = np.asarray(proj_w, np.float32)
    rpe = np.asarray(rpe_table, np.float32)

    wq64 = np.concatenate([qw[0:C].T, qb[None, 0:C]], 0) * scale     # [65, 64]
    wk64 = np.concatenate([qw[C:2 * C].T, qb[None, C:2 * C]], 0)     # [65, 64]
    wq = np.zeros((C + 1, 128), np.float32)
    wk = np.zeros((C + 1, 128), np.float32)
    for h in range(H):
        wq[:, 32 * h:32 * h + D] = wq64[:, D * h:D * (h + 1)]
        wk[:, 32 * h:32 * h + D] = wk64[:, D * h:D * (h + 1)]
    wv = np.zeros((C + 1, H * C), np.float32)
    for h in range(H):
        vh = qw[2 * C + h * D: 2 * C + (h + 1) * D]                  # [16, 64]
        bh = qb[2 * C + h * D: 2 * C + (h + 1) * D]
        ph = pw[:, h * D:(h + 1) * D]                                # [64, 16]
        wv[0:C, h * C:(h + 1) * C] = vh.T @ ph.T
        wv[C, h * C:(h + 1) * C] = bh @ ph.T

    u, v = np.arange(GMAX)[:, None], np.arange(GMAX)[None, :]
    duv = np.clip(u - v, -POS_BND, POS_BND) + POS_BND
    mtab = np.zeros((128, 2 * H * GMAX), np.float32)
    for h in range(H):
        mtab[0:64, h * GMAX:(h + 1) * GMAX] = rpe[duv, h]
        mtab[64:128, h * GMAX:(h + 1) * GMAX] = rpe[duv + RPE_NUM, h]
        mtab[0:64, K + h * GMAX:K + (h + 1) * GMAX] = rpe[duv + 2 * RPE_NUM, h]

    pb = np.broadcast_to(np.asarray(proj_b, np.float32), (128, C)).copy()

    iota64 = np.arange(GMAX)
    in_maps = []
    for c in range(NCORES):
        rs = slice(c * NR, (c + 1) * NR)
        ft = np.ones((C + 1, NR), np.float32)
        ft[0:C] = f_ser[rs].T
        gc_c = gc_ser[rs]                                            # [NR, 3]
        u01 = np.zeros((2 * GMAX, NR), np.float32)
        u01[0:GMAX] = gc_c[:, 0][None, :] == iota64[:, None]
        u01[GMAX:] = gc_c[:, 1][None, :] == iota64[:, None]
        u2b = np.zeros((GMAX + NB, NR), np.float32)
        u2b[0:GMAX] = gc_c[:, 2][None, :] == iota64[:, None]
        u2b[GMAX:] = MASK * (b_ser[rs][None, :] == np.arange(NB)[:, None])
        # DoubleRow lhsT: partition v: (U0[v], U2[v]); 64+v: (U1[v], mask[v])
        udr = np.zeros((128, 2, NR), np.float32)
        udr[0:GMAX, 0] = u01[0:GMAX]
        udr[0:GMAX, 1] = u2b[0:GMAX]
        udr[GMAX:, 0] = u01[GMAX:]
        udr[GMAX:GMAX + NB, 1] = u2b[GMAX:]
        sel = np.zeros((NB, GMAX), np.float32)
        sel[np.arange(NB), np.arange(NB)] = 1.0
        f8 = ml_dtypes.float8_e4m3fn
        in_maps.append({
            "ft": ft.astype(bf), "u01": u01.astype(bf), "u2b": u2b.astype(bf),
            "udr": udr.reshape(128, 2 * NR).astype(f8), "sel": sel.astype(bf),
            "mtab": mtab.astype(bf), "wq": wq.astype(bf), "wk": wk.astype(bf),
            "wv": wv.astype(bf), "pb": pb.astype(bf),
        })
    return in_maps


def kernel(feat, qkv_w, qkv_b, proj_w, proj_b, rpe_table,
           order, inverse, grid_coord, batch, num_batches, _state={}):
    in_maps = _host_prep(feat, qkv_w, qkv_b, proj_w, proj_b, rpe_table,
                         order, grid_coord, batch, num_batches)
    if "nc" not in _state:
        _state["nc"] = _build_program()
    res = run_bass_kernel_spmd(_state["nc"], in_maps, list(range(NCORES)))
    y_ser = np.concatenate([np.asarray(r["z"]) for r in res.results], 0)
    out = np.empty((N, C), np.float32)
    out[np.asarray(order)] = y_ser[:N]
    return out


# revision 58
# speedup vs baseline: 1.0989x; 1.0368x over previous
"""PointTransformerV3 windowed sparse attention on 8 Trainium2 cores.

Strategy: shard the P=256 serialized windows across 8 cores (32 windows
each).  The host does the serialization gather (feat[order]) when
building each core's shard; everything else runs on-device:

  - qkv projection (bias folded via an appended ones-row of f^T)
  - RPE bias: for each axis a and head h the 64x64 table
    M_ah[u, v] = rpe[clip(u - v)] turns the lookup into two chained
    matmuls against one-hot coordinate encodings:
        S^T[j, i] += onehot(gc_j)^T @ (M_ah @ onehot(gc_i))
    which lands the bias directly in the attention-logit PSUM
    accumulation (no [K,K,H] gather is ever materialized in HBM).
  - batch mask: +32*onehot5(b_j) . 32*onehot5(b_i) adds 1024 to
    same-batch logits; exp is taken as exp(S - 1024) so cross-batch
    pairs underflow to exactly 0 like the reference's -1000 mask.
  - P @ V~ with proj folded into V~ and an appended ones column to get
    the softmax denominator for free; per-partition reciprocal scale.
"""

import numpy as np
import ml_dtypes

import concourse.bass as bass
import concourse.bacc as bacc
import concourse.mybir as mybir
from concourse.tile import TileContext
from concourse.bass_utils import run_bass_kernel_spmd

BF16 = mybir.dt.bfloat16
FP8 = mybir.dt.float8e4
F32 = mybir.dt.float32

N, C, H, K, D = 65432, 64, 4, 256, 16
POS_BND, RPE_NUM = 20, 41
PAD = (K - N % K) % K          # 104
NPAD = N + PAD                 # 65536
P = NPAD // K                  # 256 windows
NCORES = 8
PC = P // NCORES               # 32 windows per core
NR = NPAD // NCORES            # 8192 rows per core
GMAX = 64                      # grid coord range [0, 64)
NB = 5                         # batch ids 0..3 plus pad id 4
MASK = 32.0                    # 32*32 = 1024 added to same-batch logits

bf = ml_dtypes.bfloat16


def _build_program():
    nc = bacc.Bacc()

    ft_d = nc.dram_tensor("ft", [C + 1, NR], BF16, kind="ExternalInput")
    u01_d = nc.dram_tensor("u01", [2 * GMAX, NR], BF16, kind="ExternalInput")
    u2b_d = nc.dram_tensor("u2b", [GMAX + NB, NR], BF16, kind="ExternalInput")
    udr_d = nc.dram_tensor("udr", [128, 2 * NR], FP8, kind="ExternalInput")
    sel_d = nc.dram_tensor("sel", [NB, GMAX], BF16, kind="ExternalInput")
    # rows 0:64 axis0 (cols 0:256) + axis2 (cols 256:512); rows 64:128 axis1
    mtab_d = nc.dram_tensor("mtab", [128, 2 * H * GMAX], BF16, kind="ExternalInput")
    # head h occupies partitions 32h..32h+16 (32-aligned for AP rules)
    wq_d = nc.dram_tensor("wq", [C + 1, 128], BF16, kind="ExternalInput")
    wk_d = nc.dram_tensor("wk", [C + 1, 128], BF16, kind="ExternalInput")
    wv_d = nc.dram_tensor("wv", [C + 1, H * C], BF16, kind="ExternalInput")
    pb_d = nc.dram_tensor("pb", [128, C], BF16, kind="ExternalInput")
    z_d = nc.dram_tensor("z", [NR, C], F32, kind="ExternalOutput")

    with TileContext(nc) as tc:
        with (
            tc.tile_pool(name="const", bufs=1) as cpool,
            tc.tile_pool(name="io", bufs=4) as iopool,
            tc.tile_pool(name="work", bufs=3) as wpool,
            tc.tile_pool(name="heads", bufs=12) as hpool,
            tc.tile_pool(name="psA", bufs=1, space="PSUM") as psA,
            tc.tile_pool(name="psB", bufs=2, space="PSUM") as psB,
        ):
            mtab_s = cpool.tile_from(mtab_d[:, :])
            wq_s = cpool.tile_from(wq_d[:, :])
            wk_s = cpool.tile_from(wk_d[:, :])
            wv_s = cpool.tile_from(wv_d[:, :])
            pb_s = cpool.tile_from(pb_d[:, :])
            sel_s = cpool.tile_from(sel_d[:, :])
            nbias = cpool.tile([128, 1], F32)
            nc.vector.memset(nbias[:, :], -1024.0)
            # double-buffered q-blockdiag tiles; zeros written once, only the
            # 16-row head blocks are rewritten each patch
            qzt = []
            for i in range(2):
                t = cpool.tile([128, H * K], BF16, name=f"qzc_{i}", tag=f"qzc{i}")
                nc.vector.memset(t[:, :], 0.0)
                qzt.append(t)

            def mt(a, h):
                if a == 0:
                    return mtab_s[0:64, h * GMAX:(h + 1) * GMAX]
                if a == 1:
                    return mtab_s[64:128, h * GMAX:(h + 1) * GMAX]
                return mtab_s[0:64, K + h * GMAX:K + (h + 1) * GMAX]

            for p in range(PC):
                sl = slice(p * K, (p + 1) * K)

                ft_s = iopool.tile([C + 1, K], BF16, name=f"ft_{p}", tag="ft")
                nc.sync.dma_start(out=ft_s[:, :], in_=ft_d[:, sl])
                u01_s = iopool.tile([128, K], BF16, name=f"u01_{p}", tag="u01")
                nc.sync.dma_start(out=u01_s[:, :], in_=u01_d[:, sl])

                # shared lhsT for the second S contraction tile:
                # rows 0:64 U2, 64:69 mask onehot (same for all heads)
                l2_s = iopool.tile([69, K], BF16, name=f"l2_{p}", tag="l2")
                nc.sync.dma_start(out=l2_s[:, :], in_=u2b_d[:, sl])
                # DR-interleaved one-hot weights for the merged S matmul
                udr_s = iopool.tile([128, 2, K], FP8, name=f"udr_{p}", tag="udr")
                nc.sync.dma_start(
                    out=udr_s[:, :, :],
                    in_=udr_d.rearrange("p (s n) -> p s n", s=2)[:, :, sl])
                ohb_s = iopool.tile([NB, K], BF16, name=f"ohb_{p}", tag="ohb")
                nc.sync.dma_start(out=ohb_s[:, :], in_=u2b_d[GMAX:GMAX + NB, sl])

                # q/k projections: kq[:, 0:256] = k^T, [:, 256:512] = q^T*scale
                kq_ps = psA.tile([128, 2 * K], F32, name=f"kq_{p}", tag="kq")
                nc.tensor.matmul(kq_ps[:, 0:K], wk_s[:, :], ft_s[:, :])
                nc.tensor.matmul(kq_ps[:, K:2 * K], wq_s[:, :], ft_s[:, :])
                kq_s = wpool.tile([128, 2 * K], BF16, name=f"kqs_{p}", tag="kqs")
                nc.scalar.copy(kq_s[:, :], kq_ps[:, :])
                # per-head q with other heads' rows zeroed: contract-128 qk
                # matmul against all-head k picks out the h block only
                qz_s = qzt[p % 2]
                for h in range(H):
                    hs = slice(h * 32, h * 32 + D)
                    nc.vector.tensor_copy(
                        qz_s[hs, h * K:(h + 1) * K], kq_s[hs, K:2 * K])

                # V~ = f @ Wv~^T laid out [j, (h, o)]
                vb_ps = psA.tile([128, 2 * K], F32, name=f"vb_{p}", tag="vb")
                for jb in range(2):
                    nc.tensor.matmul(
                        vb_ps[:, jb * K:(jb + 1) * K],
                        ft_s[:, jb * 128:(jb + 1) * 128],
                        wv_s[:, :],
                    )
                # vs[j, jb, h*65:h*65+64] = V~_h ; col h*65+64 = ones
                vs_s = wpool.tile([128, 2, H * 65], BF16, name=f"vs_{p}", tag="vs")
                for jb in range(2):
                    nc.vector.tensor_copy(
                        vs_s[:, jb, :].rearrange("p (h x) -> p h x", h=H)[:, :, 0:C],
                        vb_ps[:, jb * K:(jb + 1) * K].rearrange(
                            "p (h x) -> p h x", h=H),
                    )
                nc.vector.memset(
                    vs_s.rearrange("p b (h x) -> p b h x", h=H)[:, :, :, C], 1.0)

                # RPE gather-equivalent, laid out for DoubleRow pairing:
                # partition v: (G0[v], G2[v]); partition 64+v: (G1[v], mask[v])
                rdr = []
                for h in range(H):
                    g_ps = psB.tile([128, 2 * K], F32, name=f"g_{p}_{h}", tag="g")
                    nc.tensor.matmul(g_ps[0:64, 0:K], mt(0, h), u01_s[0:64, :])
                    nc.tensor.matmul(g_ps[64:128, 0:K], mt(1, h), u01_s[64:128, :])
                    nc.tensor.matmul(g_ps[0:64, K:2 * K], mt(2, h), l2_s[0:64, :])
                    nc.tensor.matmul(g_ps[64:128, K:2 * K], sel_s[:, :], ohb_s[:, :])
                    rd = hpool.tile([128, 2 * K], FP8, name=f"rdr_{p}_{h}", tag="rdr")
                    if h < 2:
                        nc.scalar.copy(rd[:, :], g_ps[:, :])
                    else:
                        nc.vector.tensor_copy(rd[:, :], g_ps[:, :])
                    rdr.append(rd)

                # attention logits S^T[j, i] per head, then P = exp(S - 1024)
                pmat = []
                for h in range(H):
                    s_ps = psB.tile([128, 2 * K], F32, name=f"s_{p}_{h}", tag="s")
                    for jb in range(2):
                        jsl = slice(jb * 128, (jb + 1) * 128)
                        osl = slice(jb * K, (jb + 1) * K)
                        nc.tensor.matmul(
                            s_ps[:, osl], udr_s[:, :, jsl],
                            rdr[h].rearrange("p (s x) -> p s x", s=2),
                            start=True, stop=False,
                            perf_mode=mybir.MatmulPerfMode.DoubleRow)
                        nc.tensor.matmul(
                            s_ps[:, osl], kq_s[:, jsl],
                            qz_s[:, h * K:(h + 1) * K],
                            start=False, stop=True)
                    pm = hpool.tile([128, 2 * K], BF16, name=f"p_{p}_{h}", tag="pm")
                    nc.scalar.activation(
                        pm[:, :], s_ps[:, :], mybir.ActivationFunctionType.Exp,
                        bias=nbias[:, :], scale=1.0)
                    pmat.append(pm)

                # u[i, h*65 : h*65+65] = [P @ V~_h | rowsum]
                ub = []
                for ib in range(2):
                    u_ps = psB.tile([128, H * 65], F32, name=f"u_{p}_{ib}", tag="u")
                    for h in range(H):
                        for jb in range(2):
                            nc.tensor.matmul(
                                u_ps[:, h * 65:(h + 1) * 65],
                                pmat[h][:, jb * K + ib * 128: jb * K + ib * 128 + 128],
                                vs_s[:, jb, h * 65:(h + 1) * 65],
                                start=(jb == 0), stop=(jb == 1))
                    ub.append(u_ps)

                # z = sum_h u_h / s_h + pb (adds on the idle gpsimd engine)
                zz = wpool.tile([128, 2, C], F32, name=f"zz_{p}", tag="zz")
                for ib in range(2):
                    r_s = wpool.tile([128, H], F32, name=f"r_{p}_{ib}", tag="r")
                    nc.vector.reciprocal(
                        r_s[:, :],
                        ub[ib].rearrange("p (h x) -> p h x", h=H)[:, :, C])
                    # one broadcast mul per head-PAIR into its own tile:
                    # each gpsimd add then depends on exactly one DVE op
                    ya = []
                    for g2 in range(2):
                        t = hpool.tile([128, 2, C], BF16,
                                       name=f"ya_{p}_{ib}_{g2}", tag=f"ya{g2}",
                                       bufs=8)
                        nc.vector.tensor_mul(
                            t[:, :, :],
                            ub[ib].rearrange("p (h x) -> p h x", h=H)[
                                :, 2 * g2:2 * g2 + 2, 0:C],
                            r_s[:, 2 * g2:2 * g2 + 2].rearrange(
                                "p (h o) -> p h o", o=1).to_broadcast(
                                (128, 2, C)))
                        ya.append(t)
                    t01 = wpool.tile([128, C], BF16, name=f"t01_{p}_{ib}", tag="t01")
                    t23 = wpool.tile([128, C], BF16, name=f"t23_{p}_{ib}", tag="t23")
                    nc.gpsimd.tensor_add(t01[:, :], ya[0][:, 0, :], ya[0][:, 1, :])
                    nc.gpsimd.tensor_add(t23[:, :], ya[1][:, 0, :], ya[1][:, 1, :])
                    t03 = wpool.tile([128, C], BF16, name=f"t03_{p}_{ib}", tag="t03")
                    nc.gpsimd.tensor_add(t03[:, :], t01[:, :], t23[:, :])
                    nc.gpsimd.tensor_add(zz[:, ib, :], t03[:, :], pb_s[:, :])
                nc.sync.dma_start(
                    out=z_d[p * K:(p + 1) * K, :].rearrange(
                        "(b i) c -> i b c", b=2),
                    in_=zz[:, :, :])
    nc.compile()
    return nc


def _host_prep(feat, qkv_w, qkv_b, proj_w, proj_b, rpe_table,
               order, grid_coord, batch, num_batches):
    scale = D ** -0.5
    order = np.asarray(order)

    f_ser = np.zeros((NPAD, C), np.float32)
    f_ser[:N] = np.asarray(feat, np.float32)[order]
    gc_ser = np.zeros((NPAD, 3), np.int64)
    gc_ser[:N] = np.asarray(grid_coord)[order]
    b_ser = np.full((NPAD,), int(num_batches), np.int64)
    b_ser[:N] = np.asarray(batch)

    # shared parameter-derived tensors
    qw = np.asarray(qkv_w, np.float32)
    qb = np.asarray(qkv_b, np.float32)
    pw = np.asarray(proj_w, np.float32)
    rpe = np.asarray(rpe_table, np.float32)

    wq64 = np.concatenate([qw[0:C].T, qb[None, 0:C]], 0) * scale     # [65, 64]
    wk64 = np.concatenate([qw[C:2 * C].T, qb[None, C:2 * C]], 0)     # [65, 64]
    wq = np.zeros((C + 1, 128), np.float32)
    wk = np.zeros((C + 1, 128), np.float32)
    for h in range(H):
        wq[:, 32 * h:32 * h + D] = wq64[:, D * h:D * (h + 1)]
        wk[:, 32 * h:32 * h + D] = wk64[:, D * h:D * (h + 1)]
    wv = np.zeros((C + 1, H * C), np.float32)
    for h in range(H):
        vh = qw[2 * C + h * D: 2 * C + (h + 1) * D]                  # [16, 64]
        bh = qb[2 * C + h * D: 2 * C + (h + 1) * D]
        ph = pw[:, h * D:(h + 1) * D]                                # [64, 16]
        wv[0:C, h * C:(h + 1) * C] = vh.T @ ph.T
        wv[C, h * C:(h + 1) * C] = bh @ ph.T

    u, v = np.arange(GMAX)[:, None], np.arange(GMAX)[None, :]
    duv = np.clip(u - v, -POS_BND, POS_BND) + POS_BND
    mtab = np.zeros((128, 2 * H * GMAX), np.float32)
    for h in range(H):
        mtab[0:64, h * GMAX:(h + 1) * GMAX] = rpe[duv, h]
        mtab[64:128, h * GMAX:(h + 1) * GMAX] = rpe[duv + RPE_NUM, h]
        mtab[0:64, K + h * GMAX:K + (h + 1) * GMAX] = rpe[duv + 2 * RPE_NUM, h]

    pb = np.broadcast_to(np.asarray(proj_b, np.float32), (128, C)).copy()

    iota64 = np.arange(GMAX)
    in_maps = []
    for c in range(NCORES):
        rs = slice(c * NR, (c + 1) * NR)
        ft = np.ones((C + 1, NR), np.float32)
        ft[0:C] = f_ser[rs].T
        gc_c = gc_ser[rs]                                            # [NR, 3]
        u01 = np.zeros((2 * GMAX, NR), np.float32)
        u01[0:GMAX] = gc_c[:, 0][None, :] == iota64[:, None]
        u01[GMAX:] = gc_c[:, 1][None, :] == iota64[:, None]
        u2b = np.zeros((GMAX + NB, NR), np.float32)
        u2b[0:GMAX] = gc_c[:, 2][None, :] == iota64[:, None]
        u2b[GMAX:] = MASK * (b_ser[rs][None, :] == np.arange(NB)[:, None])
        # DoubleRow lhsT: partition v: (U0[v], U2[v]); 64+v: (U1[v], mask[v])
        udr = np.zeros((128, 2, NR), np.float32)
        udr[0:GMAX, 0] = u01[0:GMAX]
        udr[0:GMAX, 1] = u2b[0:GMAX]
        udr[GMAX:, 0] = u01[GMAX:]
        udr[GMAX:GMAX + NB, 1] = u2b[GMAX:]
        sel = np.zeros((NB, GMAX), np.float32)
        sel[np.arange(NB), np.arange(NB)] = 1.0
        f8 = ml_dtypes.float8_e4m3fn
        in_maps.append({
            "ft": ft.astype(bf), "u01": u01.astype(bf), "u2b": u2b.astype(bf),
            "udr": udr.reshape(128, 2 * NR).astype(f8), "sel": sel.astype(bf),
            "mtab": mtab.astype(bf), "wq": wq.astype(bf), "wk": wk.astype(bf),
            "wv": wv.astype(bf), "pb": pb.astype(bf),
        })
    return in_maps


def kernel(feat, qkv_w, qkv_b, proj_w, proj_b, rpe_table,
           order, inverse, grid_coord, batch, num_batches, _state={}):
    in_maps = _host_prep(feat, qkv_w, qkv_b, proj_w, proj_b, rpe_table,
                         order, grid_coord, batch, num_batches)
    if "nc" not in _state:
        _state["nc"] = _build_program()
    res = run_bass_kernel_spmd(_state["nc"], in_maps, list(range(NCORES)))
    y_ser = np.concatenate([np.asarray(r["z"]) for r in res.results], 0)
    out = np.empty((N, C), np.float32)
    out[np.asarray(order)] = y_ser[:N]
    return out
